# revision 1
# baseline (speedup 1.0000x reference)
"""BigARDecoder Trainium2 kernel: data-parallel over batch (B=64 -> 8 rows/core),
folded weights, hidden-major compute, streamed inner-cell weights.

Host-side algebraic fold (exact in fp32) halves per-step weight traffic:
  inner cell i: h1 feeds only the Whh matmul  =>  W*_i = Wih_i + Whh_i @ hA_W_i
  out cell:     hh feeds only out_Whh         =>  Wout* = out_Wih + out_Whh@lastH_W
  last_c path:  cc = c2(cell2) @ (lastC_W@cA_W[3]).T + fold bias
Everything device-side is hidden-major ([unit-tiles x batch]) with weights as
the stationary matmul operand -> no transposes anywhere. bf16 weights/acts,
fp32 psum + elementwise (validated rel_l2 ~9e-3 vs reference).

Residency: in-cell + out-cell weights live in SBUF; the four inner cells'
(W* | cA) k-tiles stream per step, triple-buffered. mt is resident.

Engines: sync = streaming + output DMA; PE = matmuls; ACT = LUTs; DVE = gate
combines + bias adds. All loop via Fori with monotone semaphore thresholds.
gpsimd is unused (collectives on this fleet cost ~200us -> pure DP).
"""
import numpy as np
import ml_dtypes

import concourse.bass as bass
import concourse.bacc as bacc
import concourse.mybir as mybir
from concourse.bass_utils import run_bass_kernel_spmd

NCORES = 8
B, T, IN, H, G, O, D = 64, 256, 256, 1024, 256, 64, 4
GO = G + O            # 320
BL = B // NCORES      # 8 batch rows per core
TS = T - 1            # 255 scan steps
MO = 384              # padded out-cell gate section
NW = 10 * D           # streamed weight chunks (4 m-tiles each) per step
NBUF = 3              # stream buffers
CW = 4096             # chunk width: 4 m-tiles x (8 k-tiles x 128)

BF = mybir.dt.bfloat16
FP = mybir.dt.float32
AF = mybir.ActivationFunctionType
ALU = mybir.AluOpType

_bf = lambda a: np.ascontiguousarray(a).astype(ml_dtypes.bfloat16)
_f32 = lambda a: np.ascontiguousarray(a, dtype=np.float32)


# --------------------------------------------------------------------------
# host-side preparation
# --------------------------------------------------------------------------

def _ktiles(wT, pad_k=None):
    """[K, M] fp32 -> [128, (K/128)*M] fp32, k-tiles along the free dim."""
    K, M = wT.shape
    if pad_k is not None and pad_k != K:
        w = np.zeros((pad_k, M), np.float32)
        w[:K] = wT
        wT, K = w, pad_k
    nk = K // 128
    return np.concatenate([wT[k * 128:(k + 1) * 128] for k in range(nk)], 1)


def _btiles(b, nb=BL):
    """[M] -> [128, (M/128)*nb] fp32, col m*nb+j = b[m*128 + p]."""
    M = b.shape[0]
    nm = M // 128
    out = np.zeros((128, nm * nb), np.float32)
    for m in range(nm):
        out[:, m * nb:(m + 1) * nb] = b[m * 128:(m + 1) * 128, None]
    return out


def prepare(inp):
    f = {k: _f32(v) for k, v in inp.items()}

    Wstar, bstar = [], []
    for i in range(D):
        Wstar.append(f["rnns_Wih"][i] + f["rnns_Whh"][i] @ f["hA_W"][i])
        bstar.append(f["rnns_bih"][i] + f["rnns_bhh"][i]
                     + f["hA_b"][i] @ f["rnns_Whh"][i].T)
    Wout = f["out_Wih"] + f["out_Whh"] @ f["lastH_W"]
    bout = f["out_bih"] + f["out_bhh"] + f["lastH_b"] @ f["out_Whh"].T
    lastC = f["lastC_W"] @ f["cA_W"][3]
    bcc = f["lastC_b"] + f["cA_b"][3] @ f["lastC_W"].T
    b_in = f["in_bih"] + f["in_bhh"]

    sh = {}
    sh["w_in"] = _bf(np.concatenate(
        [_ktiles(f["in_Wih"].T, pad_k=MO), _ktiles(f["in_Whh"].T)], 1))
    for d in range(D):
        wt = _ktiles(Wstar[d].T)          # [128, 8kt*4096], m-tile m at kt*4096+m*128
        ct = _ktiles(f["cA_W"][d].T)      # [128, 8kt*1024]
        cols = []
        for mi in range(40):              # 32 gate m-tiles then 8 cA m-tiles
            for kt in range(8):
                if mi < 32:
                    cols.append(wt[:, kt * 4096 + mi * 128:kt * 4096 + (mi + 1) * 128])
                else:
                    mj = mi - 32
                    cols.append(ct[:, kt * 1024 + mj * 128:kt * 1024 + (mj + 1) * 128])
        sh[f"w_s{d}"] = _bf(np.concatenate(cols, 1))   # [128, 40*1024]
    wo = np.zeros((4 * MO, H), np.float32)
    bo = np.zeros((4 * MO,), np.float32)
    for g in range(4):
        wo[g * MO:g * MO + GO] = Wout[g * GO:(g + 1) * GO]
        bo[g * MO:g * MO + GO] = bout[g * GO:(g + 1) * GO]
    sh["w_out"] = _bf(_ktiles(wo.T))
    wc = np.zeros((MO, H), np.float32)
    bc = np.zeros((MO,), np.float32)
    wc[:GO] = lastC
    bc[:GO] = bcc
    sh["w_cc"] = _bf(_ktiles(wc.T))

    sh["bias_in"] = np.concatenate(
        [_btiles(b_in[g * H:(g + 1) * H]) for g in range(4)], 1)
    for d in range(D):
        bb = [_btiles(bstar[d][g * H:(g + 1) * H]) for g in range(4)]
        bb.append(_btiles(f["cA_b"][d]))
        sh[f"bias_c{d}"] = np.concatenate(bb, 1)
    bb = [_btiles(bo[g * MO:(g + 1) * MO]) for g in range(4)]
    bb.append(_btiles(bc))
    sh["bias_out"] = np.concatenate(bb, 1)              # [128, 5*24]

    for pre in ("eh", "ec", "ex"):
        sh[pre + "1"] = _bf(_ktiles(f[pre + "_W1"].T))
        sh[pre + "2"] = _bf(_ktiles(f[pre + "_W2"].T))
        sh[pre + "3"] = _bf(_ktiles(f[pre + "_W3"].T))
        sh[pre + "b1"] = _btiles(f[pre + "_b1"])
        sh[pre + "b2"] = _btiles(f[pre + "_b2"])
        if pre == "ex":
            b3 = np.zeros((128, BL), np.float32)
            b3[:O] = f[pre + "_b3"][:, None]
            sh[pre + "b3"] = b3
        else:
            sh[pre + "b3"] = _btiles(f[pre + "_b3"])

    in_maps = []
    for c in range(NCORES):
        m = dict(sh)
        bs = slice(c * BL, (c + 1) * BL)
        # mt resident layout [128, T*3*BL]: col t*24 + kt*8 + b
        mt = np.zeros((128, T * 3 * BL), np.float32)
        blk = f["m_true"][bs, ::-1, :]      # [BL, T, GO]; index t = T-1-t'
        for kt in range(3):
            lo, hi = kt * 128, min((kt + 1) * 128, GO)
            # [BL, T, hi-lo] -> [p, t, b]
            v = blk[:, :, lo:hi].transpose(2, 1, 0)   # [hi-lo, T, BL]
            for t in range(T):
                mt[:hi - lo, t * (3 * BL) + kt * BL:(t * (3 * BL) + (kt + 1) * BL)] = v[:, t, :]
        m["mt"] = _bf(mt)
        zt = np.zeros((128, 2 * BL), np.float32)
        zT = f["z"][bs].T
        zt[:, :BL] = zT[:128]
        zt[:, BL:] = zT[128:]
        m["zt"] = _bf(zt)
        in_maps.append(m)
    return in_maps


# --------------------------------------------------------------------------
# device program
# --------------------------------------------------------------------------

def build(ts=TS):
    nc = bacc.Bacc()
    E = nc.declare_dram_parameter

    w_in_e = E("w_in", [128, 11 * 4096], BF, isOutput=False)
    w_s_e = [E(f"w_s{d}", [128, 10 * CW], BF, isOutput=False) for d in range(D)]
    w_out_e = E("w_out", [128, 8 * 1536], BF, isOutput=False)
    w_cc_e = E("w_cc", [128, 8 * 384], BF, isOutput=False)
    bias_in_e = E("bias_in", [128, 4 * 64], FP, isOutput=False)
    bias_c_e = [E(f"bias_c{d}", [128, 5 * 64], FP, isOutput=False) for d in range(D)]
    bias_out_e = E("bias_out", [128, 5 * 24], FP, isOutput=False)
    mlp_e = {}
    for pre in ("eh", "ec", "ex"):
        n3 = 8 * 1024 if pre != "ex" else 8 * 64
        mlp_e[pre + "1"] = E(pre + "1", [128, 2 * 1024], BF, isOutput=False)
        mlp_e[pre + "2"] = E(pre + "2", [128, 8 * 1024], BF, isOutput=False)
        mlp_e[pre + "3"] = E(pre + "3", [128, n3], BF, isOutput=False)
        mlp_e[pre + "b1"] = E(pre + "b1", [128, 8 * BL], FP, isOutput=False)
        mlp_e[pre + "b2"] = E(pre + "b2", [128, 8 * BL], FP, isOutput=False)
        mlp_e[pre + "b3"] = E(pre + "b3", [128, (8 * BL) if pre != "ex" else BL],
                              FP, isOutput=False)
    mt_e = E("mt", [128, T * 3 * BL], BF, isOutput=False)
    zt_e = E("zt", [128, 2 * BL], BF, isOutput=False)
    out_e = E("out", [T, GO, BL], FP, isOutput=True)
    xl_e = E("xlast", [O, BL], FP, isOutput=True)

    # ld counts: initial loads then 2 per mlp phase
    NLOAD = 3 + 1 + D + 1 + 2   # w_in,w_out,w_cc, bias_in, bias_c x4, bias_out, mt,zt

    from contextlib import ExitStack
    ctx = ExitStack()
    with ctx:
        block = ctx.enter_context(nc.Block())
        EC = ctx.enter_context
        ld = EC(nc.semaphore("ld")); ws = EC(nc.semaphore("ws"))
        pw = EC(nc.semaphore("pw")); ov = EC(nc.semaphore("ov"))
        od = EC(nc.semaphore("od")); pmlp = EC(nc.semaphore("pmlp"))
        dmlp = EC(nc.semaphore("dmlp")); pa = EC(nc.semaphore("pa"))
        pb = EC(nc.semaphore("pb"))
        w_in_s = EC(nc.sbuf_tensor("w_in_s", [128, 11 * 4096], BF))
        w_out_s = EC(nc.sbuf_tensor("w_out_s", [128, 8 * 1536], BF))
        w_cc_s = EC(nc.sbuf_tensor("w_cc_s", [128, 8 * 384], BF))
        wbuf = EC(nc.sbuf_tensor("wbuf", [128, NBUF * CW], BF))
        bias_in_s = EC(nc.sbuf_tensor("bias_in_s", [128, 4 * 64], FP))
        bias_c_s = EC(nc.sbuf_tensor("bias_c_s", [128, D * 5 * 64], FP))
        bias_out_s = EC(nc.sbuf_tensor("bias_out_s", [128, 5 * 24], FP))
        mt_s = EC(nc.sbuf_tensor("mt_s", [128, T * 3 * BL], BF))
        zt_s = EC(nc.sbuf_tensor("zt_s", [128, 2 * BL], BF))
        mlpw = EC(nc.sbuf_tensor("mlpw", [128, 8 * 1024], BF))
        mlpb = EC(nc.sbuf_tensor("mlpb", [128, 8 * BL], FP))
        hid = EC(nc.sbuf_tensor("hid", [128, 8 * BL], BF))
        hid2 = EC(nc.sbuf_tensor("hid2", [128, 8 * BL], BF))
        hb0 = EC(nc.sbuf_tensor("hb0", [128, 8 * BL], BF))
        hb1 = EC(nc.sbuf_tensor("hb1", [128, 8 * BL], BF))
        hb2 = EC(nc.sbuf_tensor("hb2", [128, 8 * BL], BF))
        hb3 = EC(nc.sbuf_tensor("hb3", [128, 8 * BL], BF))
        hb4 = EC(nc.sbuf_tensor("hb4", [128, 8 * BL], BF))
        cb0 = EC(nc.sbuf_tensor("cb0", [128, 8 * BL], BF))
        cb1 = EC(nc.sbuf_tensor("cb1", [128, 8 * BL], BF))
        cb2 = EC(nc.sbuf_tensor("cb2", [128, 8 * BL], BF))
        cb3 = EC(nc.sbuf_tensor("cb3", [128, 8 * BL], BF))
        cf = EC(nc.sbuf_tensor("cf", [128, 8 * BL], FP))
        tmp = EC(nc.sbuf_tensor("tmp", [128, 10 * 64], FP))
        tmpo = EC(nc.sbuf_tensor("tmpo", [128, 7 * 24], FP))
        ost = EC(nc.sbuf_tensor("ost", [128, 3 * BL], FP))
        xst = EC(nc.sbuf_tensor("xst", [128, BL], FP))
        psA = EC(nc.psum_tensor("psA", [128, 512], FP))
        psB = EC(nc.psum_tensor("psB", [128, 512], FP))
        psC = EC(nc.psum_tensor("psC", [128, 512], FP))
        psD = EC(nc.psum_tensor("psD", [128, 512], FP))
        es = [nc.semaphore(f"es{r}").__enter__() for r in range(5)]
        ps = [nc.semaphore(f"ps{r}").__enter__() for r in range(6)]
        db = [nc.semaphore(f"db{r}").__enter__() for r in range(6)]
        asm = [nc.semaphore(f"as{r}").__enter__() for r in range(6)]
        dsm = [nc.semaphore(f"ds{r}").__enter__() for r in range(6)]
        a2m = [nc.semaphore(f"a2{r}").__enter__() for r in range(6)]

        hbuf = [hb0, hb1, hb2, hb3, hb4]
        cbuf = [cb0, cb1, cb2, cb3]

        # mlp phase shapes: (nk, nm, w_mstride)
        PH = []
        for pre in ("eh", "ec", "ex"):
            PH.append((pre, 0, 2, 8, 1024))
            PH.append((pre, 1, 8, 8, 1024))
            PH.append((pre, 2, 8, 8 if pre != "ex" else 1, 1024 if pre != "ex" else 64))

        # ------------------------------ sync -------------------------------
        @block.sync
        def _(sync):
            sync.dma_start(out=w_in_s[:, :], in_=w_in_e[:, :]).then_inc(ld, 16)
            sync.dma_start(out=w_out_s[:, :], in_=w_out_e[:, :]).then_inc(ld, 16)
            sync.dma_start(out=w_cc_s[:, :], in_=w_cc_e[:, :]).then_inc(ld, 16)
            sync.dma_start(out=bias_in_s[:, :], in_=bias_in_e[:, :]).then_inc(ld, 16)
            for d in range(D):
                sync.dma_start(out=bias_c_s[:, d * 320:(d + 1) * 320],
                               in_=bias_c_e[d][:, :]).then_inc(ld, 16)
            sync.dma_start(out=bias_out_s[:, :], in_=bias_out_e[:, :]).then_inc(ld, 16)
            sync.dma_start(out=mt_s[:, :], in_=mt_e[:, :]).then_inc(ld, 16)
            sync.dma_start(out=zt_s[:, :], in_=zt_e[:, :]).then_inc(ld, 16)
            # mlp layer streams
            for ph, (pre, li, nk, nm, mstride) in enumerate(PH):
                if ph >= 1:
                    sync.wait_ge(pmlp, ph)        # PE consumed previous layer
                    sync.wait_ge(dmlp, ph)        # DVE consumed previous biases
                wlen = nk * nm * (128 if mstride == 1024 else 64)
                sync.dma_start(out=mlpw[:, 0:nk * mstride],
                               in_=mlp_e[pre + str(li + 1)].ap()).then_inc(ld, 16)
                blen = nm * BL
                sync.dma_start(out=mlpb[:, 0:blen],
                               in_=mlp_e[pre + "b" + str(li + 1)].ap()).then_inc(ld, 16)

            sync.wait_ge(dmlp, 9)
            sync.dma_start(out=xl_e[:, :], in_=xst[0:O, :]).then_inc(od, 16)
            with sync.Fori(0, ts) as t:
                for d in range(D):
                    for ci in range(10):
                        g = d * 10 + ci
                        sync.wait_ge(pw, t * NW + g + NBUF)   # serialized: 1 chunk in flight
                        bslot = g % NBUF
                        sync.dma_start(
                            out=wbuf[:, bslot * CW:(bslot + 1) * CW],
                            in_=w_s_e[d][:, ci * CW:(ci + 1) * CW],
                        ).then_inc(ws, 16)
                sync.wait_ge(ov, t)
                for m in range(3):
                    hi = 128 if m < 2 else 64
                    sync.dma_start(
                        out=out_e[bass.ds(t, 1), m * 128:m * 128 + hi, :],
                        in_=ost[0:hi, m * BL:(m + 1) * BL],
                    ).then_inc(od, 16)
            sync.wait_ge(ov, ts)
            for m in range(3):
                hi = 128 if m < 2 else 64
                sync.dma_start(
                    out=out_e[ts:ts + 1, m * 128:m * 128 + hi, :],
                    in_=ost[0:hi, m * BL:(m + 1) * BL],
                ).then_inc(od, 16)
        # ------------------------------ PE ---------------------------------
        @block.tensor
        def _(pe):
            pe.sem_inc(pw, NBUF)     # pre-seed stream-buffer credit
            # prologue MLPs
            for ph, (pre, li, nk, nm, mstride) in enumerate(PH):
                pe.wait_ge(ld, 16 * (NLOAD + 2 * (ph + 1)))
                if ph > 0:
                    pe.wait_ge(dmlp, ph)
                S = psA if ph % 2 == 0 else psB
                mm = None
                for m in range(nm):
                    for kt in range(nk):
                        rhs = (zt_s[:, kt * BL:(kt + 1) * BL] if li == 0 else
                               (hid if li == 1 else hid2)[:, kt * BL:(kt + 1) * BL])
                        if mstride == 1024:
                            w = mlpw[:, kt * mstride + m * 128:kt * mstride + (m + 1) * 128]
                            o = S[:, m * BL:(m + 1) * BL]
                        else:
                            w = mlpw[:, kt * 64:(kt + 1) * 64]
                            o = S[0:64, m * BL:(m + 1) * BL]
                        mm = pe.matmul(o, w, rhs,
                                       start=(kt == 0), stop=(kt == nk - 1))
                mm.then_inc(pmlp, 1)

            with pe.Fori(0, ts) as t:
                # R_in (psC): kt 0-2 m_t, 3-10 h
                pe.wait_ge(es[4], t + 1)
                mm = None
                for m in range(32):
                    for kt in range(11):
                        rhs = (mt_s[:, bass.ds(t * (3 * BL) + kt * BL, BL)] if kt < 3
                               else hb4[:, (kt - 3) * BL:(kt - 2) * BL])
                        mm = pe.matmul(
                            psC[:, m * BL:(m + 1) * BL],
                            w_in_s[:, kt * 4096 + m * 128:kt * 4096 + (m + 1) * 128],
                            rhs, start=(kt == 0), stop=(kt == 10))
                mm.then_inc(ps[0], 1)
                # cells d=0..3
                for d in range(D):
                    S = psA if d % 2 == 0 else psB
                    rh = hbuf[d]
                    rc = cbuf[d]
                    pe.wait_ge(es[d], t + 1)
                    for ci in range(10):
                        g = d * 10 + ci
                        pe.wait_ge(ws, (t * NW + g + 1) * 16)
                        wb = wbuf[:, (g % NBUF) * CW:(g % NBUF + 1) * CW]
                        for mi in range(4):
                            m = ci * 4 + mi
                            src_t = rh if m < 32 else rc
                            for kt in range(8):
                                mm = pe.matmul(
                                    S[:, m * BL:(m + 1) * BL],
                                    wb[:, (mi * 8 + kt) * 128:(mi * 8 + kt + 1) * 128],
                                    src_t[:, kt * BL:(kt + 1) * BL],
                                    start=(kt == 0), stop=(kt == 7))
                        mm.then_inc(pw, 1)
                # R_out (psD): gates on hb4(cell3 h), cc on cb3(cell2 c)
                pe.wait_ge(es[4], t + 2)
                for m in range(12):
                    for kt in range(8):
                        mm = pe.matmul(
                            psD[:, m * BL:(m + 1) * BL],
                            w_out_s[:, kt * 1536 + m * 128:kt * 1536 + (m + 1) * 128],
                            hb4[:, kt * BL:(kt + 1) * BL],
                            start=(kt == 0), stop=(kt == 7))
                for m in range(3):
                    for kt in range(8):
                        mm = pe.matmul(
                            psD[:, (12 + m) * BL:(13 + m) * BL],
                            w_cc_s[:, kt * 384 + m * 128:kt * 384 + (m + 1) * 128],
                            cb3[:, kt * BL:(kt + 1) * BL],
                            start=(kt == 0), stop=(kt == 7))
                mm.then_inc(ps[5], 1)

        # ------------------------------ DVE --------------------------------
        @block.vector
        def _(dve):
            # prologue elementwise
            for ph, (pre, li, nk, nm, mstride) in enumerate(PH):
                S = psA if ph % 2 == 0 else psB
                w = nm * BL
                dve.wait_ge(pmlp, ph + 1)
                if li < 2:
                    dst = hid if li == 0 else hid2
                    a = dve.tensor_add(tmp[:, 0:w], S[:, 0:w], mlpb[:, 0:w])
                    a.then_inc(pa, 1)
                    dve.wait_ge(pb, ph + 1)
                    dve.tensor_tensor(dst[:, 0:w], tmp[:, 0:w], tmp[:, 64:64 + w],
                                      ALU.mult).then_inc(dmlp, 1)
                else:
                    if pre == "eh":
                        a = dve.tensor_add(hb4[:, 0:w], S[:, 0:w], mlpb[:, 0:w])
                    elif pre == "ec":
                        a = dve.tensor_add(cf[:, 0:w], S[:, 0:w], mlpb[:, 0:w])
                    else:
                        a = dve.tensor_add(xst[:, 0:BL], S[:, 0:BL], mlpb[:, 0:BL])
                    a.then_inc(dmlp, 1)
                    dve.sem_inc(pa, 1)
            dve.memset(ost[:, :], 0.0)
            dve.tensor_copy(tmp[0:1, 0:1], tmp[0:1, 1:2]).then_inc(es[4], 1)

            SL0 = [tmp[:, i * 64:(i + 1) * 64] for i in range(10)]

            def rnd(r, t, S, bias, wv, c_from_psum, c_psum_off, dst_h, dst_c,
                    cn_to_cf, to_ost):
                sl = [x[:, 0:wv] for x in SL0]
                if 1 <= r <= 4:
                    dve.wait_ge(pw, NBUF + t * NW + r * 10)
                else:
                    dve.wait_ge(ps[0 if r == 0 else 5], t + 1)
                for gi in range(4):
                    a = dve.tensor_add(sl[gi], S[:, gi * wv:(gi + 1) * wv],
                                       bias[:, gi * wv:(gi + 1) * wv])
                if c_from_psum:
                    a = dve.tensor_add(sl[8], S[:, c_psum_off:c_psum_off + wv],
                                       bias[:, 4 * wv:5 * wv])
                a.then_inc(db[r], 1)
                dve.wait_ge(asm[r], t + 1)
                dve.tensor_tensor(sl[0], sl[4], sl[6], ALU.mult)      # m1=si*tg
                c_in = sl[8] if c_from_psum else cf[:, :]
                dve.tensor_tensor(sl[1], sl[5], c_in, ALU.mult)       # m2=sf*c
                cn_dst = cf[:, :] if cn_to_cf else sl[9]
                dve.tensor_tensor(cn_dst, sl[0], sl[1], ALU.add).then_inc(dsm[r], 1)
                dve.wait_ge(a2m[r], t + 1)
                if to_ost:
                    dve.wait_ge(od, 16 + 48 * (t + 1))
                    dve.tensor_tensor(ost[:, 0:wv], sl[7], sl[8],
                                      ALU.mult).then_inc(ov, 1)
                else:
                    h = dve.tensor_tensor(dst_h[:, :], sl[7], sl[8], ALU.mult)
                    if dst_c is not None:
                        dve.tensor_copy(dst_c[:, :], sl[9]).then_inc(es[r], 1)
                    else:
                        h.then_inc(es[r], 1)

            with dve.Fori(0, ts) as t:
                rnd(0, t, psC, bias_in_s, 64, False, 0, hb0, cb0, False, False)
                for d in range(D):
                    S = psA if d % 2 == 0 else psB
                    rnd(1 + d, t, S, bias_c_s[:, d * 320:(d + 1) * 320], 64,
                        True, 256, hbuf[d + 1], (cbuf[d + 1] if d < 3 else None),
                        d == 3, False)
                rnd(5, t, psD, bias_out_s, 24, True, 96, None, None, False, True)

        # ------------------------------ ACT --------------------------------
        @block.scalar
        def _(act):
            # prologue: sigmoids for SiLU
            for ph, (pre, li, nk, nm, mstride) in enumerate(PH):
                w = nm * BL
                act.wait_ge(pa, ph + 1)
                if li < 2:
                    act.activation(tmp[:, 64:64 + w], tmp[:, 0:w],
                                   AF.Sigmoid).then_inc(pb, 1)
                else:
                    act.sem_inc(pb, 1)

            SL0 = [tmp[:, i * 64:(i + 1) * 64] for i in range(10)]

            def arnd(r, t, wv, tc_from_cf):
                sl = [x[:, 0:wv] for x in SL0]
                act.wait_ge(db[r], t + 1)
                act.activation(sl[6], sl[2], AF.Tanh)       # tg
                act.activation(sl[4], sl[0], AF.Sigmoid)    # si
                act.activation(sl[5], sl[1], AF.Sigmoid)    # sf
                act.activation(sl[7], sl[3], AF.Sigmoid).then_inc(asm[r], 1)  # so
                act.wait_ge(dsm[r], t + 1)
                src = cf[:, :] if tc_from_cf else sl[9]
                act.activation(sl[8], src, AF.Tanh).then_inc(a2m[r], 1)  # tc

            with act.Fori(0, ts) as t:
                arnd(0, t, 64, False)
                for d in range(D):
                    arnd(1 + d, t, 64, d == 3)
                arnd(5, t, 24, False)

    return nc


# --------------------------------------------------------------------------
# host entry
# --------------------------------------------------------------------------

_CACHE = {}


def _get_nc(ts):
    if ts not in _CACHE:
        nc = build(ts)
        nc.compile()
        _CACHE[ts] = nc
    return _CACHE[ts]


def run_device(inputs, ts=TS):
    in_maps = prepare(inputs)
    nc = _get_nc(ts)
    res = run_bass_kernel_spmd(nc, in_maps, core_ids=list(range(NCORES)))
    return res


def assemble(res, ts=TS):
    m_hat = np.zeros((B, T, GO), np.float32)
    for c in range(NCORES):
        o = res.results[c]["out"]               # [T, GO, BL]
        m_hat[c * BL:(c + 1) * BL] = o.transpose(2, 0, 1)
    xl = res.results[0]["xlast"]                # [O, BL] of core 0 only!
    m_hat[:, 0, :] = 0.0
    for c in range(NCORES):
        xlc = res.results[c]["xlast"]           # [O, BL]
        m_hat[c * BL:(c + 1) * BL, 0, G:] = xlc.T
    return m_hat


def kernel(**inputs):
    res = run_device(inputs, TS)
    return assemble(res, TS)



# revision 3
# speedup vs baseline: 5.0865x; 5.0865x over previous
"""BigARDecoder Trainium2 kernel: data-parallel over batch (B=64 -> 8 rows/core),
folded weights, hidden-major compute, streamed inner-cell weights.

Host-side algebraic fold (exact in fp32) halves per-step weight traffic:
  inner cell i: h1 feeds only the Whh matmul  =>  W*_i = Wih_i + Whh_i @ hA_W_i
  out cell:     hh feeds only out_Whh         =>  Wout* = out_Wih + out_Whh@lastH_W
  last_c path:  cc = c2(cell2) @ (lastC_W@cA_W[3]).T + fold bias
Everything device-side is hidden-major ([unit-tiles x batch]) with weights as
the stationary matmul operand -> no transposes anywhere. bf16 weights/acts,
fp32 psum + elementwise (validated rel_l2 ~9e-3 vs reference).

Residency: in-cell + out-cell weights live in SBUF; the four inner cells'
(W* | cA) k-tiles stream per step, triple-buffered. mt is resident.

Engines: sync = streaming + output DMA; PE = matmuls; ACT = LUTs; DVE = gate
combines + bias adds. All loop via Fori with monotone semaphore thresholds.
gpsimd is unused (collectives on this fleet cost ~200us -> pure DP).
"""
import numpy as np
import ml_dtypes

import concourse.bass as bass
import concourse.bacc as bacc
import concourse.mybir as mybir
from concourse.bass_utils import run_bass_kernel_spmd

NCORES = 8
B, T, IN, H, G, O, D = 64, 256, 256, 1024, 256, 64, 4
GO = G + O            # 320
BL = B // NCORES      # 8 batch rows per core
TS = T - 1            # 255 scan steps
MO = 384              # padded out-cell gate section
NW = 10 * D           # streamed weight chunks (4 m-tiles each) per step
NBUF = 4              # stream buffers (40 chunks/step % NBUF == 0)
CW = 4096             # chunk width: 4 m-tiles x (8 k-tiles x 128)

BF = mybir.dt.bfloat16
FP = mybir.dt.float32
AF = mybir.ActivationFunctionType
ALU = mybir.AluOpType

_bf = lambda a: np.ascontiguousarray(a).astype(ml_dtypes.bfloat16)
_f32 = lambda a: np.ascontiguousarray(a, dtype=np.float32)


# --------------------------------------------------------------------------
# host-side preparation
# --------------------------------------------------------------------------

def _ktiles(wT, pad_k=None):
    """[K, M] fp32 -> [128, (K/128)*M] fp32, k-tiles along the free dim."""
    K, M = wT.shape
    if pad_k is not None and pad_k != K:
        w = np.zeros((pad_k, M), np.float32)
        w[:K] = wT
        wT, K = w, pad_k
    nk = K // 128
    return np.concatenate([wT[k * 128:(k + 1) * 128] for k in range(nk)], 1)


def _btiles(b, nb=BL):
    """[M] -> [128, (M/128)*nb] fp32, col m*nb+j = b[m*128 + p]."""
    M = b.shape[0]
    nm = M // 128
    out = np.zeros((128, nm * nb), np.float32)
    for m in range(nm):
        out[:, m * nb:(m + 1) * nb] = b[m * 128:(m + 1) * 128, None]
    return out


def prepare(inp):
    f = {k: _f32(v) for k, v in inp.items()}

    Wstar, bstar = [], []
    for i in range(D):
        Wstar.append(f["rnns_Wih"][i] + f["rnns_Whh"][i] @ f["hA_W"][i])
        bstar.append(f["rnns_bih"][i] + f["rnns_bhh"][i]
                     + f["hA_b"][i] @ f["rnns_Whh"][i].T)
    Wout = f["out_Wih"] + f["out_Whh"] @ f["lastH_W"]
    bout = f["out_bih"] + f["out_bhh"] + f["lastH_b"] @ f["out_Whh"].T
    lastC = f["lastC_W"] @ f["cA_W"][3]
    bcc = f["lastC_b"] + f["cA_b"][3] @ f["lastC_W"].T
    b_in = f["in_bih"] + f["in_bhh"]

    sh = {}
    sh["w_in"] = _bf(np.concatenate(
        [_ktiles(f["in_Wih"].T, pad_k=MO), _ktiles(f["in_Whh"].T)], 1))
    for d in range(D):
        wt = _ktiles(Wstar[d].T)          # [128, 8kt*4096], m-tile m at kt*4096+m*128
        ct = _ktiles(f["cA_W"][d].T)      # [128, 8kt*1024]
        cols = []
        for mi in range(40):              # 32 gate m-tiles then 8 cA m-tiles
            for kt in range(8):
                if mi < 32:
                    cols.append(wt[:, kt * 4096 + mi * 128:kt * 4096 + (mi + 1) * 128])
                else:
                    mj = mi - 32
                    cols.append(ct[:, kt * 1024 + mj * 128:kt * 1024 + (mj + 1) * 128])
        sh[f"w_s{d}"] = _bf(np.concatenate(cols, 1))   # [128, 40*1024]
    wo = np.zeros((4 * MO, H), np.float32)
    bo = np.zeros((4 * MO,), np.float32)
    for g in range(4):
        wo[g * MO:g * MO + GO] = Wout[g * GO:(g + 1) * GO]
        bo[g * MO:g * MO + GO] = bout[g * GO:(g + 1) * GO]
    sh["w_out"] = _bf(_ktiles(wo.T))
    wc = np.zeros((MO, H), np.float32)
    bc = np.zeros((MO,), np.float32)
    wc[:GO] = lastC
    bc[:GO] = bcc
    sh["w_cc"] = _bf(_ktiles(wc.T))

    sh["bias_in"] = np.concatenate(
        [_btiles(b_in[g * H:(g + 1) * H]) for g in range(4)], 1)
    for d in range(D):
        bb = [_btiles(bstar[d][g * H:(g + 1) * H]) for g in range(4)]
        bb.append(_btiles(f["cA_b"][d]))
        sh[f"bias_c{d}"] = np.concatenate(bb, 1)
    bb = [_btiles(bo[g * MO:(g + 1) * MO]) for g in range(4)]
    bb.append(_btiles(bc))
    sh["bias_out"] = np.concatenate(bb, 1)              # [128, 5*24]

    for pre in ("eh", "ec", "ex"):
        sh[pre + "1"] = _bf(_ktiles(f[pre + "_W1"].T))
        sh[pre + "2"] = _bf(_ktiles(f[pre + "_W2"].T))
        sh[pre + "3"] = _bf(_ktiles(f[pre + "_W3"].T))
        sh[pre + "b1"] = _btiles(f[pre + "_b1"])
        sh[pre + "b2"] = _btiles(f[pre + "_b2"])
        if pre == "ex":
            b3 = np.zeros((128, BL), np.float32)
            b3[:O] = f[pre + "_b3"][:, None]
            sh[pre + "b3"] = b3
        else:
            sh[pre + "b3"] = _btiles(f[pre + "_b3"])

    in_maps = []
    for c in range(NCORES):
        m = dict(sh)
        bs = slice(c * BL, (c + 1) * BL)
        # mt resident layout [128, T*3*BL]: col t*24 + kt*8 + b
        mt = np.zeros((128, T * 3 * BL), np.float32)
        blk = f["m_true"][bs, ::-1, :]      # [BL, T, GO]; index t = T-1-t'
        for kt in range(3):
            lo, hi = kt * 128, min((kt + 1) * 128, GO)
            # [BL, T, hi-lo] -> [p, t, b]
            v = blk[:, :, lo:hi].transpose(2, 1, 0)   # [hi-lo, T, BL]
            for t in range(T):
                mt[:hi - lo, t * (3 * BL) + kt * BL:(t * (3 * BL) + (kt + 1) * BL)] = v[:, t, :]
        m["mt"] = _bf(mt)
        zt = np.zeros((128, 2 * BL), np.float32)
        zT = f["z"][bs].T
        zt[:, :BL] = zT[:128]
        zt[:, BL:] = zT[128:]
        m["zt"] = _bf(zt)
        in_maps.append(m)
    return in_maps


# --------------------------------------------------------------------------
# device program
# --------------------------------------------------------------------------

def build(ts=TS):
    nc = bacc.Bacc()
    E = nc.declare_dram_parameter

    w_in_e = E("w_in", [128, 11 * 4096], BF, isOutput=False)
    w_s_e = [E(f"w_s{d}", [128, 10 * CW], BF, isOutput=False) for d in range(D)]
    w_out_e = E("w_out", [128, 8 * 1536], BF, isOutput=False)
    w_cc_e = E("w_cc", [128, 8 * 384], BF, isOutput=False)
    bias_in_e = E("bias_in", [128, 4 * 64], FP, isOutput=False)
    bias_c_e = [E(f"bias_c{d}", [128, 5 * 64], FP, isOutput=False) for d in range(D)]
    bias_out_e = E("bias_out", [128, 5 * 24], FP, isOutput=False)
    mlp_e = {}
    for pre in ("eh", "ec", "ex"):
        n3 = 8 * 1024 if pre != "ex" else 8 * 64
        mlp_e[pre + "1"] = E(pre + "1", [128, 2 * 1024], BF, isOutput=False)
        mlp_e[pre + "2"] = E(pre + "2", [128, 8 * 1024], BF, isOutput=False)
        mlp_e[pre + "3"] = E(pre + "3", [128, n3], BF, isOutput=False)
        mlp_e[pre + "b1"] = E(pre + "b1", [128, 8 * BL], FP, isOutput=False)
        mlp_e[pre + "b2"] = E(pre + "b2", [128, 8 * BL], FP, isOutput=False)
        mlp_e[pre + "b3"] = E(pre + "b3", [128, (8 * BL) if pre != "ex" else BL],
                              FP, isOutput=False)
    mt_e = E("mt", [128, T * 3 * BL], BF, isOutput=False)
    zt_e = E("zt", [128, 2 * BL], BF, isOutput=False)
    out_e = E("out", [T, GO, BL], FP, isOutput=True)
    xl_e = E("xlast", [O, BL], FP, isOutput=True)

    # ld counts: initial loads then 2 per mlp phase
    NLOAD = 3 + 1 + D + 1 + 2   # w_in,w_out,w_cc, bias_in, bias_c x4, bias_out, mt,zt

    from contextlib import ExitStack
    ctx = ExitStack()
    with ctx:
        block = ctx.enter_context(nc.Block())
        EC = ctx.enter_context
        ld = EC(nc.semaphore("ld"))
        wsv = [EC(nc.semaphore(f"ws{i}")) for i in range(4)]
        pw = EC(nc.semaphore("pw")); ov = EC(nc.semaphore("ov"))
        od = EC(nc.semaphore("od")); pmlp = EC(nc.semaphore("pmlp"))
        dmlp = EC(nc.semaphore("dmlp")); pa = EC(nc.semaphore("pa"))
        pb = EC(nc.semaphore("pb"))
        w_in_s = EC(nc.sbuf_tensor("w_in_s", [128, 11 * 4096], BF))
        w_out_s = EC(nc.sbuf_tensor("w_out_s", [128, 8 * 1536], BF))
        w_cc_s = EC(nc.sbuf_tensor("w_cc_s", [128, 8 * 384], BF))
        wbuf = EC(nc.sbuf_tensor("wbuf", [128, NBUF * CW], BF))
        bias_in_s = EC(nc.sbuf_tensor("bias_in_s", [128, 4 * 64], FP))
        bias_c_s = EC(nc.sbuf_tensor("bias_c_s", [128, D * 5 * 64], FP))
        bias_out_s = EC(nc.sbuf_tensor("bias_out_s", [128, 5 * 24], FP))
        mt_s = EC(nc.sbuf_tensor("mt_s", [128, T * 3 * BL], BF))
        zt_s = EC(nc.sbuf_tensor("zt_s", [128, 2 * BL], BF))
        mlpw = EC(nc.sbuf_tensor("mlpw", [128, 8 * 1024], BF))
        mlpb = EC(nc.sbuf_tensor("mlpb", [128, 8 * BL], FP))
        hid = EC(nc.sbuf_tensor("hid", [128, 8 * BL], BF))
        hid2 = EC(nc.sbuf_tensor("hid2", [128, 8 * BL], BF))
        hb0 = EC(nc.sbuf_tensor("hb0", [128, 8 * BL], BF))
        hb1 = EC(nc.sbuf_tensor("hb1", [128, 8 * BL], BF))
        hb2 = EC(nc.sbuf_tensor("hb2", [128, 8 * BL], BF))
        hb3 = EC(nc.sbuf_tensor("hb3", [128, 8 * BL], BF))
        hb4 = EC(nc.sbuf_tensor("hb4", [128, 8 * BL], BF))
        cb0 = EC(nc.sbuf_tensor("cb0", [128, 8 * BL], BF))
        cb1 = EC(nc.sbuf_tensor("cb1", [128, 8 * BL], BF))
        cb2 = EC(nc.sbuf_tensor("cb2", [128, 8 * BL], BF))
        cb3 = EC(nc.sbuf_tensor("cb3", [128, 8 * BL], BF))
        cf = EC(nc.sbuf_tensor("cf", [128, 8 * BL], FP))
        tmp = EC(nc.sbuf_tensor("tmp", [128, 10 * 64], FP))
        tmpo = EC(nc.sbuf_tensor("tmpo", [128, 7 * 24], FP))
        ost = EC(nc.sbuf_tensor("ost", [128, 3 * BL], FP))
        xst = EC(nc.sbuf_tensor("xst", [128, BL], FP))
        psA = EC(nc.psum_tensor("psA", [128, 512], FP))
        psB = EC(nc.psum_tensor("psB", [128, 512], FP))
        psC = EC(nc.psum_tensor("psC", [128, 512], FP))
        psD = EC(nc.psum_tensor("psD", [128, 512], FP))
        es = [nc.semaphore(f"es{r}").__enter__() for r in range(5)]
        ps = [nc.semaphore(f"ps{r}").__enter__() for r in range(6)]
        db = [nc.semaphore(f"db{r}").__enter__() for r in range(6)]
        asm = [nc.semaphore(f"as{r}").__enter__() for r in range(6)]
        dsm = [nc.semaphore(f"ds{r}").__enter__() for r in range(6)]
        a2m = [nc.semaphore(f"a2{r}").__enter__() for r in range(6)]

        hbuf = [hb0, hb1, hb2, hb3, hb4]
        cbuf = [cb0, cb1, cb2, cb3]

        # mlp phase shapes: (nk, nm, w_mstride)
        PH = []
        for pre in ("eh", "ec", "ex"):
            PH.append((pre, 0, 2, 8, 1024))
            PH.append((pre, 1, 8, 8, 1024))
            PH.append((pre, 2, 8, 8 if pre != "ex" else 1, 1024 if pre != "ex" else 64))

        # ------------------------------ sync -------------------------------
        @block.sync
        def _(sync):
            sync.dma_start(out=w_in_s[:, :], in_=w_in_e[:, :]).then_inc(ld, 16)
            sync.dma_start(out=w_out_s[:, :], in_=w_out_e[:, :]).then_inc(ld, 16)
            sync.dma_start(out=w_cc_s[:, :], in_=w_cc_e[:, :]).then_inc(ld, 16)
            sync.dma_start(out=bias_in_s[:, :], in_=bias_in_e[:, :]).then_inc(ld, 16)
            for d in range(D):
                sync.dma_start(out=bias_c_s[:, d * 320:(d + 1) * 320],
                               in_=bias_c_e[d][:, :]).then_inc(ld, 16)
            sync.dma_start(out=bias_out_s[:, :], in_=bias_out_e[:, :]).then_inc(ld, 16)
            sync.dma_start(out=mt_s[:, :], in_=mt_e[:, :]).then_inc(ld, 16)
            sync.dma_start(out=zt_s[:, :], in_=zt_e[:, :]).then_inc(ld, 16)
            # mlp layer streams
            for ph, (pre, li, nk, nm, mstride) in enumerate(PH):
                if ph >= 1:
                    sync.wait_ge(pmlp, ph)        # PE consumed previous layer
                    sync.wait_ge(dmlp, ph)        # DVE consumed previous biases
                wlen = nk * nm * (128 if mstride == 1024 else 64)
                sync.dma_start(out=mlpw[:, 0:nk * mstride],
                               in_=mlp_e[pre + str(li + 1)].ap()).then_inc(ld, 16)
                blen = nm * BL
                sync.dma_start(out=mlpb[:, 0:blen],
                               in_=mlp_e[pre + "b" + str(li + 1)].ap()).then_inc(ld, 16)

            sync.wait_ge(dmlp, 9)
            sync.dma_start(out=xl_e[:, :], in_=xst[0:O, :]).then_inc(od, 16)
            with sync.Fori(0, ts) as t:
                for d in range(D):
                    for ci in range(10):
                        g = d * 10 + ci
                        sync.wait_ge(pw, t * NW + g + 1)   # slot g%NBUF free
                        bslot = g % NBUF
                        sync.dma_start(
                            out=wbuf[:, bslot * CW:(bslot + 1) * CW],
                            in_=w_s_e[d][:, ci * CW:(ci + 1) * CW],
                        ).then_inc(wsv[bslot], 16)
                sync.wait_ge(ov, t)
                for m in range(3):
                    hi = 128 if m < 2 else 64
                    sync.dma_start(
                        out=out_e[bass.ds(t, 1), m * 128:m * 128 + hi, :],
                        in_=ost[0:hi, m * BL:(m + 1) * BL],
                    ).then_inc(od, 16)
            sync.wait_ge(ov, ts)
            for m in range(3):
                hi = 128 if m < 2 else 64
                sync.dma_start(
                    out=out_e[ts:ts + 1, m * 128:m * 128 + hi, :],
                    in_=ost[0:hi, m * BL:(m + 1) * BL],
                ).then_inc(od, 16)
        # ------------------------------ PE ---------------------------------
        @block.tensor
        def _(pe):
            pe.sem_inc(pw, NBUF)     # pre-seed stream-buffer credit
            # prologue MLPs
            for ph, (pre, li, nk, nm, mstride) in enumerate(PH):
                pe.wait_ge(ld, 16 * (NLOAD + 2 * (ph + 1)))
                if ph > 0:
                    pe.wait_ge(dmlp, ph)
                S = psA if ph % 2 == 0 else psB
                mm = None
                for m in range(nm):
                    for kt in range(nk):
                        rhs = (zt_s[:, kt * BL:(kt + 1) * BL] if li == 0 else
                               (hid if li == 1 else hid2)[:, kt * BL:(kt + 1) * BL])
                        if mstride == 1024:
                            w = mlpw[:, kt * mstride + m * 128:kt * mstride + (m + 1) * 128]
                            o = S[:, m * BL:(m + 1) * BL]
                        else:
                            w = mlpw[:, kt * 64:(kt + 1) * 64]
                            o = S[0:64, m * BL:(m + 1) * BL]
                        mm = pe.matmul(o, w, rhs,
                                       start=(kt == 0), stop=(kt == nk - 1))
                mm.then_inc(pmlp, 1)

            with pe.Fori(0, ts) as t:
                # R_in (psC): kt 0-2 m_t, 3-10 h
                pe.wait_ge(es[4], t + 1)
                mm = None
                for m in range(32):
                    for kt in range(11):
                        rhs = (mt_s[:, bass.ds(t * (3 * BL) + kt * BL, BL)] if kt < 3
                               else hb4[:, (kt - 3) * BL:(kt - 2) * BL])
                        mm = pe.matmul(
                            psC[:, m * BL:(m + 1) * BL],
                            w_in_s[:, kt * 4096 + m * 128:kt * 4096 + (m + 1) * 128],
                            rhs, start=(kt == 0), stop=(kt == 10))
                mm.then_inc(ps[0], 1)
                # cells d=0..3
                for d in range(D):
                    S = psA if d % 2 == 0 else psB
                    rh = hbuf[d]
                    rc = cbuf[d]
                    pe.wait_ge(es[d], t + 1)
                    for ci in range(10):
                        g = d * 10 + ci
                        pe.wait_ge(wsv[g % NBUF], (t * 10 + g // 4 + 1) * 16)
                        wb = wbuf[:, (g % NBUF) * CW:(g % NBUF + 1) * CW]
                        for mi in range(4):
                            m = ci * 4 + mi
                            src_t = rh if m < 32 else rc
                            for kt in range(8):
                                mm = pe.matmul(
                                    S[:, m * BL:(m + 1) * BL],
                                    wb[:, (mi * 8 + kt) * 128:(mi * 8 + kt + 1) * 128],
                                    src_t[:, kt * BL:(kt + 1) * BL],
                                    start=(kt == 0), stop=(kt == 7))
                        mm.then_inc(pw, 1)
                # R_out (psD): gates on hb4(cell3 h), cc on cb3(cell2 c)
                pe.wait_ge(es[4], t + 2)
                for m in range(12):
                    for kt in range(8):
                        mm = pe.matmul(
                            psD[:, m * BL:(m + 1) * BL],
                            w_out_s[:, kt * 1536 + m * 128:kt * 1536 + (m + 1) * 128],
                            hb4[:, kt * BL:(kt + 1) * BL],
                            start=(kt == 0), stop=(kt == 7))
                for m in range(3):
                    for kt in range(8):
                        mm = pe.matmul(
                            psD[:, (12 + m) * BL:(13 + m) * BL],
                            w_cc_s[:, kt * 384 + m * 128:kt * 384 + (m + 1) * 128],
                            cb3[:, kt * BL:(kt + 1) * BL],
                            start=(kt == 0), stop=(kt == 7))
                mm.then_inc(ps[5], 1)

        # ------------------------------ DVE --------------------------------
        @block.vector
        def _(dve):
            # prologue elementwise
            for ph, (pre, li, nk, nm, mstride) in enumerate(PH):
                S = psA if ph % 2 == 0 else psB
                w = nm * BL
                dve.wait_ge(pmlp, ph + 1)
                if li < 2:
                    dst = hid if li == 0 else hid2
                    a = dve.tensor_add(tmp[:, 0:w], S[:, 0:w], mlpb[:, 0:w])
                    a.then_inc(pa, 1)
                    dve.wait_ge(pb, ph + 1)
                    dve.tensor_tensor(dst[:, 0:w], tmp[:, 0:w], tmp[:, 64:64 + w],
                                      ALU.mult).then_inc(dmlp, 1)
                else:
                    if pre == "eh":
                        a = dve.tensor_add(hb4[:, 0:w], S[:, 0:w], mlpb[:, 0:w])
                    elif pre == "ec":
                        a = dve.tensor_add(cf[:, 0:w], S[:, 0:w], mlpb[:, 0:w])
                    else:
                        a = dve.tensor_add(xst[:, 0:BL], S[:, 0:BL], mlpb[:, 0:BL])
                    a.then_inc(dmlp, 1)
                    dve.sem_inc(pa, 1)
            dve.memset(ost[:, :], 0.0)
            dve.tensor_copy(tmp[0:1, 0:1], tmp[0:1, 1:2]).then_inc(es[4], 1)

            SL0 = [tmp[:, i * 64:(i + 1) * 64] for i in range(10)]

            def rnd(r, t, S, bias, wv, c_from_psum, c_psum_off, dst_h, dst_c,
                    cn_to_cf, to_ost):
                sl = [x[:, 0:wv] for x in SL0]
                if 1 <= r <= 4:
                    dve.wait_ge(pw, NBUF + t * NW + r * 10)
                else:
                    dve.wait_ge(ps[0 if r == 0 else 5], t + 1)
                for gi in range(4):
                    a = dve.tensor_add(sl[gi], S[:, gi * wv:(gi + 1) * wv],
                                       bias[:, gi * wv:(gi + 1) * wv])
                if c_from_psum:
                    a = dve.tensor_add(sl[8], S[:, c_psum_off:c_psum_off + wv],
                                       bias[:, 4 * wv:5 * wv])
                a.then_inc(db[r], 1)
                dve.wait_ge(asm[r], t + 1)
                dve.tensor_tensor(sl[0], sl[4], sl[6], ALU.mult)      # m1=si*tg
                c_in = sl[8] if c_from_psum else cf[:, :]
                dve.tensor_tensor(sl[1], sl[5], c_in, ALU.mult)       # m2=sf*c
                cn_dst = cf[:, :] if cn_to_cf else sl[9]
                dve.tensor_tensor(cn_dst, sl[0], sl[1], ALU.add).then_inc(dsm[r], 1)
                dve.wait_ge(a2m[r], t + 1)
                if to_ost:
                    dve.wait_ge(od, 16 + 48 * (t + 1))
                    dve.tensor_tensor(ost[:, 0:wv], sl[7], sl[8],
                                      ALU.mult).then_inc(ov, 1)
                else:
                    h = dve.tensor_tensor(dst_h[:, :], sl[7], sl[8], ALU.mult)
                    if dst_c is not None:
                        dve.tensor_copy(dst_c[:, :], sl[9]).then_inc(es[r], 1)
                    else:
                        h.then_inc(es[r], 1)

            with dve.Fori(0, ts) as t:
                rnd(0, t, psC, bias_in_s, 64, False, 0, hb0, cb0, False, False)
                for d in range(D):
                    S = psA if d % 2 == 0 else psB
                    rnd(1 + d, t, S, bias_c_s[:, d * 320:(d + 1) * 320], 64,
                        True, 256, hbuf[d + 1], (cbuf[d + 1] if d < 3 else None),
                        d == 3, False)
                rnd(5, t, psD, bias_out_s, 24, True, 96, None, None, False, True)

        # ------------------------------ ACT --------------------------------
        @block.scalar
        def _(act):
            # prologue: sigmoids for SiLU
            for ph, (pre, li, nk, nm, mstride) in enumerate(PH):
                w = nm * BL
                act.wait_ge(pa, ph + 1)
                if li < 2:
                    act.activation(tmp[:, 64:64 + w], tmp[:, 0:w],
                                   AF.Sigmoid).then_inc(pb, 1)
                else:
                    act.sem_inc(pb, 1)

            SL0 = [tmp[:, i * 64:(i + 1) * 64] for i in range(10)]

            def arnd(r, t, wv, tc_from_cf):
                sl = [x[:, 0:wv] for x in SL0]
                act.wait_ge(db[r], t + 1)
                act.activation(sl[6], sl[2], AF.Tanh)       # tg
                act.activation(sl[4], sl[0], AF.Sigmoid)    # si
                act.activation(sl[5], sl[1], AF.Sigmoid)    # sf
                act.activation(sl[7], sl[3], AF.Sigmoid).then_inc(asm[r], 1)  # so
                act.wait_ge(dsm[r], t + 1)
                src = cf[:, :] if tc_from_cf else sl[9]
                act.activation(sl[8], src, AF.Tanh).then_inc(a2m[r], 1)  # tc

            with act.Fori(0, ts) as t:
                arnd(0, t, 64, False)
                for d in range(D):
                    arnd(1 + d, t, 64, d == 3)
                arnd(5, t, 24, False)

    return nc


# --------------------------------------------------------------------------
# host entry
# --------------------------------------------------------------------------

_CACHE = {}


def _get_nc(ts):
    if ts not in _CACHE:
        nc = build(ts)
        nc.compile()
        _CACHE[ts] = nc
    return _CACHE[ts]


def run_device(inputs, ts=TS):
    in_maps = prepare(inputs)
    nc = _get_nc(ts)
    res = run_bass_kernel_spmd(nc, in_maps, core_ids=list(range(NCORES)))
    return res


def assemble(res, ts=TS):
    m_hat = np.zeros((B, T, GO), np.float32)
    for c in range(NCORES):
        o = res.results[c]["out"]               # [T, GO, BL]
        m_hat[c * BL:(c + 1) * BL] = o.transpose(2, 0, 1)
    xl = res.results[0]["xlast"]                # [O, BL] of core 0 only!
    m_hat[:, 0, :] = 0.0
    for c in range(NCORES):
        xlc = res.results[c]["xlast"]           # [O, BL]
        m_hat[c * BL:(c + 1) * BL, 0, G:] = xlc.T
    return m_hat


def kernel(**inputs):
    res = run_device(inputs, TS)
    return assemble(res, TS)

